# revision 22
# baseline (speedup 1.0000x reference)
"""DPQ (gumbel-softmax product-quantizer autoencoder) forward pass on 8
Trainium2 NeuronCores, data-parallel over the batch dimension N.

Math (per row n, subspace m of 8, codebook of K=512 64-dim codes):
  h = x @ W_enc + b_enc                     [N, M*DSUB]
  score = (-|h|^2 + 2 h.c - |c|^2) / T_m    squared-distance scores
  codes = softmax(score + gumbel)           gumbel-softmax, TAU=1
  y = (codes @ C).flatten() @ W_dec + b_dec

Implementation notes:
  * -|h|^2 is constant over k, softmax cancels it exactly: never computed.
    z = (2 h.c - |c|^2)/T + g is bounded (~+45 max here), exp() stays in
    fp32 range, so no max-subtraction either.
  * Everything runs k-on-partitions / n-on-free.  All layout changes happen
    host-side (x, gumbel and y are passed/returned transposed, gumbel cast
    to fp16), so the device never transposes anything: the four matmul
    stages (encoder, scores, soft-lookup, decoder) are 96 PE matmuls per
    512-row block.  Gumbel noise is accumulated into the score PSUM groups
    half by fp16 identity-matmuls on the PE, half by DVE adds (K_PE=2
    balances the two engines; 4=all-PE and 1 both measured slower).
  * The codebook carried into the soft-lookup matmul is [C_m | ones*64]:
    output rows 0:63 are the unnormalized recon, rows 64:127 all hold the
    softmax denominator s, so normalization is one approx-reciprocal and
    one multiply on DVE per subspace - no broadcast matmul, no 4us exact
    DVE reciprocal.  (The s rows are first copied to a partition-0-based
    tile: the custom-DVE approx-reciprocal misreads inputs whose base
    partition differs from the output's.)
  * Score path (x, W_enc, h, 2*invT*C^T) runs in fp32r (11-bit mantissa,
    1 PE cycle/row, fp32 bits fed raw and rounded on read) - exp amplifies
    absolute score error, so 16-bit floats are not safe there.  The
    post-softmax path (codes, [C|1], recon, W_dec) runs in bf16.
    End-to-end error lands ~2.8e-3 vs the 2e-2 gate.
"""

import sys
sys.path.insert(0, '/opt/trn_rl_repo')

import numpy as np

N, D, M, K, DSUB = 32768, 512, 8, 512, 64
NCORES = 8
NLOC = N // NCORES          # rows per core
BLK = 512                   # rows per block
JC = D // 128               # 4 column chunks of 128
KC = K // 128               # 4 code chunks of 128
NT = BLK // 128

_CACHE = {}

# How many of the 4 per-subspace gumbel tiles are added by PE identity
# matmuls (the rest use DVE tensor_tensor adds into PSUM).
K_PE = 2


def build(nblk: int):
    import concourse.bacc as bacc_mod
    import concourse.tile as tile
    import concourse.mybir as mybir
    from concourse.bass import ts
    from concourse.masks import make_identity
    from contextlib import ExitStack

    F32 = mybir.dt.float32
    F32R = mybir.dt.float32r
    F16 = mybir.dt.float16
    BF16 = mybir.dt.bfloat16
    AF = mybir.ActivationFunctionType
    ALU = mybir.AluOpType

    nloc = nblk * BLK
    nc = bacc_mod.Bacc(trn_type="TRN2", target_bir_lowering=False, debug=False)

    XT = nc.dram_tensor("x_t", [D, nloc], F32R, kind="ExternalInput").ap()
    WENC = nc.dram_tensor("w_enc", [D, D], F32R, kind="ExternalInput").ap()
    BENC = nc.dram_tensor("b_enc", [D], F32, kind="ExternalInput").ap()
    CB = nc.dram_tensor("codebook", [M, K, DSUB], F32, kind="ExternalInput").ap()
    CBT = nc.dram_tensor("codebook_t", [M, DSUB, K], F32R,
                         kind="ExternalInput").ap()
    LOGT = nc.dram_tensor("log_t", [1, M], F32, kind="ExternalInput").ap()
    GUM = nc.dram_tensor("gumbel_t", [M, K, nloc], F16, kind="ExternalInput").ap()
    WDEC = nc.dram_tensor("w_dec", [D, D], F32R, kind="ExternalInput").ap()
    BDEC = nc.dram_tensor("b_dec", [D], F32, kind="ExternalInput").ap()
    YT = nc.dram_tensor("y_t", [D, nloc], F32, kind="ExternalOutput").ap()

    X4 = XT.rearrange("(dc p) (b n) -> b p dc n", p=128, n=BLK)
    G4 = GUM.rearrange("m (kc p) (b n) -> b m p kc n", p=128, n=BLK)
    Y4 = YT.rearrange("(jc p) (b n) -> b p jc n", p=128, n=BLK)

    with tile.TileContext(nc) as tc, ExitStack() as ctx:
        cst = ctx.enter_context(tc.tile_pool(name="cst", bufs=1))
        sb = ctx.enter_context(tc.tile_pool(name="sb", bufs=2))
        ps = ctx.enter_context(tc.tile_pool(name="ps", bufs=2, space="PSUM"))

        xt_t, gt_t, hr_t, cd_t, rt_t = {}, {}, {}, {}, {}

        def xt_dma(b):
            if b >= nblk:
                return
            t = sb.tile([128, JC, BLK], F32R, tag="xt", bufs=2, name="xt_t")
            nc.sync.dma_start(t[:], X4[b])
            xt_t[b] = t

        def gt_dma(u):
            if u >= nblk * M:
                return
            b, m = divmod(u, M)
            t = sb.tile([128, KC, BLK], F16, tag="gt", bufs=5, name="gt_t")
            nc.sync.dma_start(t[:], G4[b, m])
            gt_t[(b, m)] = t

        def enc(b):
            if b >= nblk:
                return
            xt = xt_t.pop(b)
            hr_t[b] = []
            for jc in range(JC):
                hp = ps.tile([128, BLK], F32, tag="hy", bufs=2, name="hp")
                for dc in range(JC):
                    nc.tensor.matmul(hp[:], lhsT=wenc[dc][:, ts(jc, 128)],
                                     rhs=xt[:, dc, :], start=(dc == 0),
                                     stop=(dc == JC - 1))
                hrt = sb.tile([128, BLK], F32R, tag=f"hr{jc}", bufs=2,
                              name="hrt")
                nc.scalar.activation(hrt[:], hp[:], AF.Identity,
                                     bias=benc_c[jc][:, 0:1], scale=1.0)
                hr_t[b].append(hrt)

        def scores(b, m):
            half = (m % 2) * 64
            gt = gt_t.pop((b, m))
            hr = hr_t[b][m // 2]
            zps = []
            for kc in range(KC):
                zp = ps.tile([128, BLK], F32, tag="zp", bufs=4, name="zp")
                nc.tensor.matmul(zp[:],
                                 lhsT=ct2p[m // 2][half:half + 64, ts(kc, 128)],
                                 rhs=hr[half:half + 64, :],
                                 start=True, stop=(kc >= K_PE),
                                 tile_position=(half, 0))
                if kc < K_PE:
                    nc.tensor.matmul(zp[:], lhsT=ident16[:], rhs=gt[:, kc, :],
                                     start=False, stop=True)
                zps.append(zp)
            cds = []
            for kc in range(KC):
                if kc >= K_PE:
                    nc.vector.tensor_tensor(zps[kc][:], zps[kc][:],
                                            gt[:, kc, :], op=ALU.add)
                cd = sb.tile([128, BLK], BF16, tag="cd", bufs=8, name="cd")
                nc.scalar.activation(cd[:], zps[kc][:], AF.Exp,
                                     bias=bias_mk[(m, kc)][:, 0:1], scale=1.0)
                cds.append(cd)
            cd_t[m] = cds

        def lookup(m, mb):
            # soft-lookup for subspace m of block mb (skewed behind scores)
            cds = cd_t.pop(m)
            up = ps.tile([128, BLK], F32, tag="up", bufs=2, name="up")
            for kc in range(KC):
                nc.tensor.matmul(up[:], lhsT=cones[(m, kc)][:], rhs=cds[kc][:],
                                 start=(kc == 0), stop=(kc == KC - 1))
            scp = sb.tile([64, BLK], F32, tag="scp", bufs=2, name="scp")
            nc.vector.tensor_copy(scp[:], up[64:128, :])
            recb = sb.tile([64, BLK], F32, tag="recb", bufs=2, name="recb")
            nc.vector.reciprocal_approx_fast(out=recb[:], in_=scp[:])
            half = (m % 2) * 64
            if m % 2 == 0:
                rt_t[m // 2] = sb.tile([128, BLK], BF16, tag=f"rt{m // 2}",
                                       bufs=2, name="rt")
            nc.vector.tensor_mul(rt_t[m // 2][half:half + 64, :],
                                 up[0:64, :], recb[:])

        def dec(b):
            yo = sb.tile([128, JC, BLK], F32, tag="yo", bufs=2, name="yo")
            for jc in range(JC):
                yp = ps.tile([128, BLK], F32, tag="hy", bufs=2, name="yp")
                for mc in range(JC):
                    nc.tensor.matmul(yp[:], lhsT=wdec[mc][:, ts(jc, 128)],
                                     rhs=rt_t[mc][:], start=(mc == 0),
                                     stop=(mc == JC - 1))
                nc.vector.tensor_scalar_add(yo[:, jc, :], yp[:],
                                            bdec_c[jc][:, 0:1])
            nc.sync.dma_start(Y4[b], yo[:])



        # ---------------- prologue: constants & weights ----------------
        ident = cst.tile([128, 128], F32, tag="ident")
        make_identity(nc, ident[:])
        ident16 = cst.tile([128, 128], F16, tag="ident16")
        nc.vector.tensor_copy(ident16[:], ident[:])
        ones_f = cst.tile([1, 128], F32, tag="ones_f")
        nc.gpsimd.memset(ones_f[:], 1.0)
        ones_r = cst.tile([1, 128], F32R, tag="ones_r")
        nc.vector.tensor_copy(ones_r[:], ones_f[:])

        # W_enc fp32r straight from DRAM; first block's x/gumbel issued
        # immediately after so the encoder can start while the rest of the
        # prologue (codebook prep, decoder weights) still streams in.
        wenc = []
        for dc in range(JC):
            wt = cst.tile([128, D], F32R, tag=f"wenc{dc}", name="wenc_t")
            nc.sync.dma_start(wt[:], WENC[ts(dc, 128), :])
            wenc.append(wt)
        xt_dma(0)
        for u in range(4):
            gt_dma(u)

        # temperatures: invT = exp(-logT) [1, 8] -> bcast to [128, 8]
        logt = cst.tile([1, M], F32, tag="logt")
        nc.sync.dma_start(logt[:], LOGT)
        invt = cst.tile([1, M], F32, tag="invt")
        nc.scalar.activation(invt[:], logt[:], AF.Exp, bias=0.0, scale=-1.0)
        invt_r = cst.tile([1, M], F32R, tag="invt_r")
        nc.vector.tensor_copy(invt_r[:], invt[:])
        ibp = ps.tile([128, M], F32, tag="hy", bufs=2, name="ibp")
        nc.tensor.matmul(ibp[:], lhsT=ones_r[:], rhs=invt_r[:],
                         start=True, stop=True)
        sc2 = cst.tile([128, M], F32, tag="sc2")    # 2*invT per partition
        nc.vector.tensor_scalar_mul(sc2[:], ibp[:], 2.0)
        scn = cst.tile([128, M], F32, tag="scn")    # -invT per partition
        nc.vector.tensor_scalar_mul(scn[:], ibp[:], -1.0)

        # encoder bias as [128,1] per column chunk (decoder's comes later)
        benc2 = BENC.rearrange("(a b) -> a b", b=1)
        bdec2 = BDEC.rearrange("(a b) -> a b", b=1)
        benc_c, bdec_c = [], []
        for jc in range(JC):
            bet = cst.tile([128, 1], F32, tag=f"benc{jc}", name="bet")
            nc.sync.dma_start(bet[:], benc2[ts(jc, 128), :])
            benc_c.append(bet)

        # per-subspace codebook prep, m ascending so scores(0,0) unblocks
        # as early as possible:
        #   ct2p[q][(m%2)*64:+64, k] = 2 invT_m C_m[d, k] fp32r (score lhsT,
        #   two subspaces stacked so rhs/lhsT partition bases line up);
        #   cones[(m,kc)] = [C_m[kc] | ones*64] bf16 (soft-lookup lhsT);
        #   bias_mk = -invT_m * |c|^2 exp bias column.
        ct2p = [cst.tile([128, K], F32R, tag=f"ct2p{q}", name="ct2p_t")
                for q in range(M // 2)]
        cones, bias_mk = {}, {}
        for m in range(M):
            half = (m % 2) * 64
            stg = sb.tile([64, K], F32R, tag="stg", bufs=4, name="stg_t")
            nc.sync.dma_start(stg[:], CBT[m])
            nc.vector.tensor_scalar_mul(ct2p[m // 2][half:half + 64, :],
                                        stg[:], sc2[0:64, m:m + 1])
            for kc in range(KC):
                chunk = sb.tile([128, DSUB], F32, tag="cchunk", bufs=6,
                                name="chunk")
                nc.sync.dma_start(chunk[:], CB[m, ts(kc, 128), :])
                scrap = sb.tile([128, DSUB], F32, tag="cscrap", bufs=2,
                                name="scrap")
                n2 = sb.tile([128, 1], F32, tag="cn2", bufs=2, name="n2")
                nc.vector.scalar_tensor_tensor(scrap[:], chunk[:], 1.0, chunk[:],
                                               op0=ALU.mult, op1=ALU.mult,
                                               accum_out=n2[:])
                bt = cst.tile([128, 1], F32, tag=f"bias{m}_{kc}", name="bt")
                nc.vector.tensor_mul(bt[:], n2[:], scn[:, m:m + 1])
                bias_mk[(m, kc)] = bt
                stage = sb.tile([128, 128], F32, tag="costg", bufs=4,
                                name="stage")
                nc.vector.tensor_copy(stage[:, 0:DSUB], chunk[:])
                nc.gpsimd.memset(stage[:, DSUB:128], 1.0)
                co = cst.tile([128, 128], BF16, tag=f"cones{m}_{kc}", name="co_t")
                nc.vector.tensor_copy(co[:], stage[:])
                cones[(m, kc)] = co

        # decoder weights (fp32 -> bf16) and bias: not needed until the
        # first dec(), so they load behind the block-0-critical DMAs.
        wdec = []
        for dc in range(JC):
            wstg = sb.tile([128, D], F32R, tag="wstg", bufs=2, name="wstg")
            nc.sync.dma_start(wstg[:], WDEC[ts(dc, 128), :])
            wd = cst.tile([128, D], BF16, tag=f"wdec{dc}", name="wdec_t")
            nc.vector.tensor_copy(wd[:], wstg[:])
            wdec.append(wd)
        for jc in range(JC):
            bdt = cst.tile([128, 1], F32, tag=f"bdec{jc}", name="bdt")
            nc.sync.dma_start(bdt[:], bdec2[ts(jc, 128), :])
            bdec_c.append(bdt)

        # ---------------- main pipelined loop over row blocks ----------------
        # Per block: enc 16mm -> per m: scores 4mm + gumbel adds (K_PE ident
        # matmuls, rest DVE), exp x4 (ACT), soft-lookup 4mm (skewed 2
        # subspaces back), s-recip + normalize (DVE) -> dec 16mm -> y out.
        # enc(b+1) is issued between up(6) and up(7) of block b to fill the
        # exp-latency bubble.
        enc(0)
        for b in range(nblk):
            for m in range(M):
                gt_dma(b * M + m + 4)
                scores(b, m)
                if m == 6:
                    xt_dma(b + 1)
                if m >= 2:
                    lookup(m - 2, b)
            lookup(6, b)
            enc(b + 1)
            lookup(7, b)
            dec(b)

    nc.compile()
    return nc


def _get_nc(nblk: int):
    key = ("nc", nblk)
    if key not in _CACHE:
        _CACHE[key] = build(nblk)
    return _CACHE[key]


def make_in_maps(inputs: dict, nblk: int):
    nloc = nblk * BLK
    xt = np.ascontiguousarray(np.asarray(inputs["x"], dtype=np.float32).T)
    gt = np.asarray(inputs["gumbel_noise"], dtype=np.float32)
    gt = gt.transpose(1, 2, 0).astype(np.float16)       # [M, K, N]
    cb = np.ascontiguousarray(inputs["codebook"], dtype=np.float32)
    shared = dict(
        w_enc=np.ascontiguousarray(inputs["W_enc"], dtype=np.float32),
        b_enc=np.ascontiguousarray(inputs["b_enc"], dtype=np.float32),
        codebook=cb,
        codebook_t=np.ascontiguousarray(cb.transpose(0, 2, 1)),
        log_t=np.ascontiguousarray(
            inputs["log_temperatures"], dtype=np.float32).reshape(1, M),
        w_dec=np.ascontiguousarray(inputs["W_dec"], dtype=np.float32),
        b_dec=np.ascontiguousarray(inputs["b_dec"], dtype=np.float32),
    )
    in_maps = []
    for c in range(NCORES):
        lo = c * NLOC
        in_maps.append(dict(
            shared,
            x_t=np.ascontiguousarray(xt[:, lo:lo + nloc]),
            gumbel_t=np.ascontiguousarray(gt[:, :, lo:lo + nloc])))
    return in_maps


def run(inputs: dict, nblk: int = NLOC // BLK, trace: bool = False):
    from concourse.bass_utils import run_bass_kernel_spmd
    nc = _get_nc(nblk)
    in_maps = make_in_maps(inputs, nblk)
    res = run_bass_kernel_spmd(nc, in_maps, list(range(NCORES)), trace=trace)
    nloc = nblk * BLK
    out = np.empty((NCORES * nloc, D), dtype=np.float32)
    for c in range(NCORES):
        out[c * nloc:(c + 1) * nloc] = res.results[c]["y_t"].T
    return out, res


def kernel(**inputs) -> np.ndarray:
    out, _ = run(inputs)
    return out


# revision 25
# speedup vs baseline: 1.0060x; 1.0060x over previous
"""DPQ (gumbel-softmax product-quantizer autoencoder) forward pass on 8
Trainium2 NeuronCores, data-parallel over the batch dimension N.

Math (per row n, subspace m of 8, codebook of K=512 64-dim codes):
  h = x @ W_enc + b_enc                     [N, M*DSUB]
  score = (-|h|^2 + 2 h.c - |c|^2) / T_m    squared-distance scores
  codes = softmax(score + gumbel)           gumbel-softmax, TAU=1
  y = (codes @ C).flatten() @ W_dec + b_dec

Implementation notes:
  * -|h|^2 is constant over k, softmax cancels it exactly: never computed.
    z = (2 h.c - |c|^2)/T + g is bounded (~+45 max here), exp() stays in
    fp32 range, so no max-subtraction either.
  * Everything runs k-on-partitions / n-on-free.  All layout changes happen
    host-side (x, gumbel and y are passed/returned transposed, gumbel cast
    to fp16), so the device never transposes anything: the four matmul
    stages (encoder, scores, soft-lookup, decoder) are 96 PE matmuls per
    512-row block.  Gumbel noise is accumulated into the score PSUM groups
    half by fp16 identity-matmuls on the PE, half by DVE adds (K_PE=2
    balances the two engines; 4=all-PE and 1 both measured slower).
  * The codebook carried into the soft-lookup matmul is [C_m | ones*64]:
    output rows 0:63 are the unnormalized recon, rows 64:127 all hold the
    softmax denominator s, so normalization is one approx-reciprocal and
    one multiply on DVE per subspace - no broadcast matmul, no 4us exact
    DVE reciprocal.  (The s rows are first copied to a partition-0-based
    tile: the custom-DVE approx-reciprocal misreads inputs whose base
    partition differs from the output's.)
  * Score path (x, W_enc, h, 2*invT*C^T) runs in fp32r (11-bit mantissa,
    1 PE cycle/row, fp32 bits fed raw and rounded on read) - exp amplifies
    absolute score error, so 16-bit floats are not safe there.  The
    post-softmax path (codes, [C|1], recon, W_dec) runs in bf16.
    End-to-end error lands ~2.8e-3 vs the 2e-2 gate.
"""

import sys
sys.path.insert(0, '/opt/trn_rl_repo')

import numpy as np

N, D, M, K, DSUB = 32768, 512, 8, 512, 64
NCORES = 8
NLOC = N // NCORES          # rows per core
BLK = 512                   # rows per block
JC = D // 128               # 4 column chunks of 128
KC = K // 128               # 4 code chunks of 128
NT = BLK // 128

_CACHE = {}

# How many of the 4 per-subspace gumbel tiles are added by PE identity
# matmuls (the rest use DVE tensor_tensor adds into PSUM).
K_PE = 2


def build(nblk: int):
    import concourse.bacc as bacc_mod
    import concourse.tile as tile
    import concourse.mybir as mybir
    from concourse.bass import ts
    from concourse.masks import make_identity
    from contextlib import ExitStack

    F32 = mybir.dt.float32
    F32R = mybir.dt.float32r
    F16 = mybir.dt.float16
    BF16 = mybir.dt.bfloat16
    AF = mybir.ActivationFunctionType
    ALU = mybir.AluOpType

    nloc = nblk * BLK
    nc = bacc_mod.Bacc(trn_type="TRN2", target_bir_lowering=False, debug=False)

    XT = nc.dram_tensor("x_t", [D, nloc], F32R, kind="ExternalInput").ap()
    WENC = nc.dram_tensor("w_enc", [D, D], F32R, kind="ExternalInput").ap()
    BENC = nc.dram_tensor("b_enc", [D], F32, kind="ExternalInput").ap()
    CB = nc.dram_tensor("codebook", [M, K, DSUB], F32, kind="ExternalInput").ap()
    CBT = nc.dram_tensor("codebook_t", [M, DSUB, K], F32R,
                         kind="ExternalInput").ap()
    LOGT = nc.dram_tensor("log_t", [1, M], F32, kind="ExternalInput").ap()
    GUM = nc.dram_tensor("gumbel_t", [M, K, nloc], F16, kind="ExternalInput").ap()
    WDEC = nc.dram_tensor("w_dec", [D, D], F32R, kind="ExternalInput").ap()
    BDEC = nc.dram_tensor("b_dec", [D], F32, kind="ExternalInput").ap()
    YT = nc.dram_tensor("y_t", [D, nloc], F32, kind="ExternalOutput").ap()

    X4 = XT.rearrange("(dc p) (b n) -> b p dc n", p=128, n=BLK)
    G4 = GUM.rearrange("m (kc p) (b n) -> b m p kc n", p=128, n=BLK)
    Y4 = YT.rearrange("(jc p) (b n) -> b p jc n", p=128, n=BLK)

    with tile.TileContext(nc) as tc, ExitStack() as ctx:
        cst = ctx.enter_context(tc.tile_pool(name="cst", bufs=1))
        sb = ctx.enter_context(tc.tile_pool(name="sb", bufs=2))
        ps = ctx.enter_context(tc.tile_pool(name="ps", bufs=2, space="PSUM"))

        xt_t, gt_t, hr_t, cd_t, rt_t = {}, {}, {}, {}, {}

        def xt_dma(b):
            if b >= nblk:
                return
            t = sb.tile([128, JC, BLK], F32R, tag="xt", bufs=2, name="xt_t")
            nc.sync.dma_start(t[:], X4[b])
            xt_t[b] = t

        def gt_dma(u):
            if u >= nblk * M:
                return
            b, m = divmod(u, M)
            t = sb.tile([128, KC, BLK], F16, tag="gt", bufs=5, name="gt_t")
            nc.sync.dma_start(t[:], G4[b, m])
            gt_t[(b, m)] = t

        def enc(b):
            if b >= nblk:
                return
            xt = xt_t.pop(b)
            hr_t[b] = []
            for jc in range(JC):
                hp = ps.tile([128, BLK], F32, tag="hy", bufs=2, name="hp")
                for dc in range(JC):
                    nc.tensor.matmul(hp[:], lhsT=wenc[dc][:, ts(jc, 128)],
                                     rhs=xt[:, dc, :], start=(dc == 0),
                                     stop=(dc == JC - 1))
                hrt = sb.tile([128, BLK], F32R, tag=f"hr{jc}", bufs=2,
                              name="hrt")
                nc.scalar.activation(hrt[:], hp[:], AF.Identity,
                                     bias=benc_c[jc][:, 0:1], scale=1.0)
                hr_t[b].append(hrt)

        def scores(b, m):
            half = (m % 2) * 64
            gt = gt_t.pop((b, m))
            hr = hr_t[b][m // 2]
            zps = []
            for kc in range(KC):
                zp = ps.tile([128, BLK], F32, tag="zp", bufs=4, name="zp")
                nc.tensor.matmul(zp[:],
                                 lhsT=ct2p[m // 2][half:half + 64, ts(kc, 128)],
                                 rhs=hr[half:half + 64, :],
                                 start=True, stop=(kc >= K_PE),
                                 tile_position=(half, 0))
                if kc < K_PE:
                    nc.tensor.matmul(zp[:], lhsT=ident16[:], rhs=gt[:, kc, :],
                                     start=False, stop=True)
                zps.append(zp)
            cds = []
            for kc in range(KC):
                if kc >= K_PE:
                    nc.vector.tensor_tensor(zps[kc][:], zps[kc][:],
                                            gt[:, kc, :], op=ALU.add)
                cd = sb.tile([128, BLK], BF16, tag="cd", bufs=8, name="cd")
                nc.scalar.activation(cd[:], zps[kc][:], AF.Exp,
                                     bias=bias_mk[(m, kc)][:, 0:1], scale=1.0)
                cds.append(cd)
            cd_t[m] = cds

        def lookup(m, mb):
            # soft-lookup for subspace m of block mb (skewed behind scores)
            cds = cd_t.pop(m)
            up = ps.tile([128, BLK], F32, tag="up", bufs=2, name="up")
            for kc in range(KC):
                nc.tensor.matmul(up[:], lhsT=cones[(m, kc)][:], rhs=cds[kc][:],
                                 start=(kc == 0), stop=(kc == KC - 1))
            scp = sb.tile([64, BLK], F32, tag="scp", bufs=2, name="scp")
            nc.vector.tensor_copy(scp[:], up[64:128, :])
            recb = sb.tile([64, BLK], F32, tag="recb", bufs=2, name="recb")
            nc.vector.reciprocal_approx_fast(out=recb[:], in_=scp[:])
            half = (m % 2) * 64
            if m % 2 == 0:
                rt_t[m // 2] = sb.tile([128, BLK], BF16, tag=f"rt{m // 2}",
                                       bufs=2, name="rt")
            nc.vector.tensor_mul(rt_t[m // 2][half:half + 64, :],
                                 up[0:64, :], recb[:])

        def dec(b):
            yo = sb.tile([128, JC, BLK], F32, tag="yo", bufs=2, name="yo")
            for jc in range(JC):
                yp = ps.tile([128, BLK], F32, tag="hy", bufs=2, name="yp")
                for mc in range(JC):
                    nc.tensor.matmul(yp[:], lhsT=wdec[mc][:, ts(jc, 128)],
                                     rhs=rt_t[mc][:], start=(mc == 0),
                                     stop=(mc == JC - 1))
                nc.vector.tensor_scalar_add(yo[:, jc, :], yp[:],
                                            bdec_c[jc][:, 0:1])
            nc.sync.dma_start(Y4[b], yo[:])



        # ---------------- prologue: constants & weights ----------------
        ident = cst.tile([128, 128], F32, tag="ident")
        make_identity(nc, ident[:])
        ident16 = cst.tile([128, 128], F16, tag="ident16")
        nc.vector.tensor_copy(ident16[:], ident[:])
        ones_f = cst.tile([1, 128], F32, tag="ones_f")
        nc.gpsimd.memset(ones_f[:], 1.0)
        ones_r = cst.tile([1, 128], F32R, tag="ones_r")
        nc.vector.tensor_copy(ones_r[:], ones_f[:])

        # W_enc fp32r straight from DRAM; W_dec loaded fp32 then cast bf16
        # (post-softmax path tolerates bf16; shorter LDWEIGHTS).
        wenc, wdec = [], []
        for dc in range(JC):
            wt = cst.tile([128, D], F32R, tag=f"wenc{dc}", name="wenc_t")
            nc.sync.dma_start(wt[:], WENC[ts(dc, 128), :])
            wenc.append(wt)
            wstg = sb.tile([128, D], F32R, tag="wstg", bufs=2, name="wstg")
            nc.sync.dma_start(wstg[:], WDEC[ts(dc, 128), :])
            wd = cst.tile([128, D], BF16, tag=f"wdec{dc}", name="wdec_t")
            nc.vector.tensor_copy(wd[:], wstg[:])
            wdec.append(wd)
        xt_dma(0)
        for u in range(4):
            gt_dma(u)

        # temperatures: invT = exp(-logT) [1, 8] -> bcast to [128, 8]
        logt = cst.tile([1, M], F32, tag="logt")
        nc.sync.dma_start(logt[:], LOGT)
        invt = cst.tile([1, M], F32, tag="invt")
        nc.scalar.activation(invt[:], logt[:], AF.Exp, bias=0.0, scale=-1.0)
        invt_r = cst.tile([1, M], F32R, tag="invt_r")
        nc.vector.tensor_copy(invt_r[:], invt[:])
        ibp = ps.tile([128, M], F32, tag="hy", bufs=2, name="ibp")
        nc.tensor.matmul(ibp[:], lhsT=ones_r[:], rhs=invt_r[:],
                         start=True, stop=True)
        sc2 = cst.tile([128, M], F32, tag="sc2")    # 2*invT per partition
        nc.vector.tensor_scalar_mul(sc2[:], ibp[:], 2.0)
        scn = cst.tile([128, M], F32, tag="scn")    # -invT per partition
        nc.vector.tensor_scalar_mul(scn[:], ibp[:], -1.0)

        # encoder / decoder biases as [128,1] per column chunk
        benc2 = BENC.rearrange("(a b) -> a b", b=1)
        bdec2 = BDEC.rearrange("(a b) -> a b", b=1)
        benc_c, bdec_c = [], []
        for jc in range(JC):
            bet = cst.tile([128, 1], F32, tag=f"benc{jc}", name="bet")
            nc.sync.dma_start(bet[:], benc2[ts(jc, 128), :])
            benc_c.append(bet)
            bdt = cst.tile([128, 1], F32, tag=f"bdec{jc}", name="bdt")
            nc.sync.dma_start(bdt[:], bdec2[ts(jc, 128), :])
            bdec_c.append(bdt)

        # score lhsT: ct2p[q][(m%2)*64:+64, k] = 2 invT_m C_m[d, k], fp32r,
        # two subspaces stacked per tile so rhs/lhsT partition bases line up.
        ct2p = [cst.tile([128, K], F32R, tag=f"ct2p{q}", name="ct2p_t")
                for q in range(M // 2)]
        for m in range(M):
            half = (m % 2) * 64
            stg = sb.tile([64, K], F32R, tag="stg", bufs=4, name="stg_t")
            nc.sync.dma_start(stg[:], CBT[m])
            nc.vector.tensor_scalar_mul(ct2p[m // 2][half:half + 64, :],
                                        stg[:], sc2[0:64, m:m + 1])

        # soft-lookup lhsT: cones[(m,kc)] = [C_m[kc] | ones*64] fp32r, and
        # exp bias column bias_mk = -invT_m * |c|^2.
        cones, bias_mk = {}, {}
        for m in range(M):
            for kc in range(KC):
                chunk = sb.tile([128, DSUB], F32, tag="cchunk", bufs=6,
                                name="chunk")
                nc.sync.dma_start(chunk[:], CB[m, ts(kc, 128), :])
                scrap = sb.tile([128, DSUB], F32, tag="cscrap", bufs=2,
                                name="scrap")
                n2 = sb.tile([128, 1], F32, tag="cn2", bufs=2, name="n2")
                nc.vector.scalar_tensor_tensor(scrap[:], chunk[:], 1.0, chunk[:],
                                               op0=ALU.mult, op1=ALU.mult,
                                               accum_out=n2[:])
                bt = cst.tile([128, 1], F32, tag=f"bias{m}_{kc}", name="bt")
                nc.vector.tensor_mul(bt[:], n2[:], scn[:, m:m + 1])
                bias_mk[(m, kc)] = bt
                stage = sb.tile([128, 128], F32, tag="costg", bufs=4,
                                name="stage")
                nc.vector.tensor_copy(stage[:, 0:DSUB], chunk[:])
                nc.gpsimd.memset(stage[:, DSUB:128], 1.0)
                co = cst.tile([128, 128], BF16, tag=f"cones{m}_{kc}", name="co_t")
                nc.vector.tensor_copy(co[:], stage[:])
                cones[(m, kc)] = co

        # ---------------- main pipelined loop over row blocks ----------------
        # Per block: enc 16mm -> per m: scores 4mm + gumbel adds (K_PE ident
        # matmuls, rest DVE), exp x4 (ACT), soft-lookup 4mm (skewed 2
        # subspaces back), s-recip + normalize (DVE) -> dec 16mm -> y out.
        # enc(b+1) is issued between up(6) and up(7) of block b to fill the
        # exp-latency bubble.
        enc(0)
        for b in range(nblk):
            for m in range(M):
                gt_dma(b * M + m + 4)
                scores(b, m)
                if m == 6:
                    xt_dma(b + 1)
                if m >= 2:
                    lookup(m - 2, b)
            lookup(6, b)
            enc(b + 1)
            lookup(7, b)
            dec(b)

    nc.compile()
    return nc


def _get_nc(nblk: int):
    key = ("nc", nblk)
    if key not in _CACHE:
        _CACHE[key] = build(nblk)
    return _CACHE[key]


def make_in_maps(inputs: dict, nblk: int):
    nloc = nblk * BLK
    xt = np.ascontiguousarray(np.asarray(inputs["x"], dtype=np.float32).T)
    gt = np.asarray(inputs["gumbel_noise"], dtype=np.float32)
    gt = gt.transpose(1, 2, 0).astype(np.float16)       # [M, K, N]
    cb = np.ascontiguousarray(inputs["codebook"], dtype=np.float32)
    shared = dict(
        w_enc=np.ascontiguousarray(inputs["W_enc"], dtype=np.float32),
        b_enc=np.ascontiguousarray(inputs["b_enc"], dtype=np.float32),
        codebook=cb,
        codebook_t=np.ascontiguousarray(cb.transpose(0, 2, 1)),
        log_t=np.ascontiguousarray(
            inputs["log_temperatures"], dtype=np.float32).reshape(1, M),
        w_dec=np.ascontiguousarray(inputs["W_dec"], dtype=np.float32),
        b_dec=np.ascontiguousarray(inputs["b_dec"], dtype=np.float32),
    )
    in_maps = []
    for c in range(NCORES):
        lo = c * NLOC
        in_maps.append(dict(
            shared,
            x_t=np.ascontiguousarray(xt[:, lo:lo + nloc]),
            gumbel_t=np.ascontiguousarray(gt[:, :, lo:lo + nloc])))
    return in_maps


def run(inputs: dict, nblk: int = NLOC // BLK, trace: bool = False):
    from concourse.bass_utils import run_bass_kernel_spmd
    nc = _get_nc(nblk)
    in_maps = make_in_maps(inputs, nblk)
    res = run_bass_kernel_spmd(nc, in_maps, list(range(NCORES)), trace=trace)
    nloc = nblk * BLK
    out = np.empty((NCORES * nloc, D), dtype=np.float32)
    for c in range(NCORES):
        out[c * nloc:(c + 1) * nloc] = res.results[c]["y_t"].T
    return out, res


def kernel(**inputs) -> np.ndarray:
    out, _ = run(inputs)
    return out
